# revision 14
# baseline (speedup 1.0000x reference)
"""GQA attention (32 heads, 8 KV groups, rope, causal) on 8 TRN2 NeuronCores.

Sharding: tensor-parallel over KV groups — core g owns KV group g
(4 query heads + 1 kv head). Wq/Wk/Wv sharded column-wise, Wo row-wise;
each core produces a partial transposed output outT=[D,T], summed and
transposed on the host.

Per-core dataflow (T=2048 tokens, D=4096, head_dim=128):
  qT[dq,T] = wq.T @ xT   (accumulated over 32 k-tiles, psum chunked by 512 tokens)
  kT likewise; rope applied on the psum->sbuf copy; v transposed via PE.
  Per i-chunk I (512 queries), head pair: S^T[j,i] = k @ q^T -> +maskbias
    -> exp (ACT);  ctx^T[d,i] += v_j^T @ P^T,  rowsum[1,i] += ones^T @ P^T (PE)
    ctxT = psum_ctx * bcast(1/rowsum)   (ACT copy, gpsimd bcast, DVE recip/mul)
  outT[e,t] = sum_h wo_h^T-tile @ ctxT_h  (wo stationary reused over 4 t-chunks).

Matmuls run in float32r (TF32-like, 1 cycle/row) with fp32 PSUM accumulation.
"""
import math

import numpy as np

import concourse.bass as bass
import concourse.tile as tile
from concourse import bacc, mybir
from concourse.bass_utils import run_bass_kernel_spmd
from concourse.masks import make_identity

F32 = mybir.dt.float32
F32R = mybir.dt.float32r

T = 2048          # tokens
D = 4096          # model dim
HD = 128          # head dim
NH = 4            # heads per core
DQ = NH * HD      # 512 q dims per core
TC = 512          # token chunk (psum free dim)
NCH = T // TC     # 4 chunks
KT = D // 128     # 32 contraction tiles
JT = T // 128     # 16 key tiles
NET = D // 128    # 32 output-row tiles (of outT)
SCALE = 1.0 / math.sqrt(HD)
NCORES = 8


def build_nc():
    nc = bacc.Bacc("TRN2", target_bir_lowering=False, debug=False, num_devices=NCORES)
    xT = nc.dram_tensor("xT", [D, T], F32, kind="ExternalInput").ap()
    wq = nc.dram_tensor("wq", [D, DQ], F32, kind="ExternalInput").ap()
    wk = nc.dram_tensor("wk", [D, HD], F32, kind="ExternalInput").ap()
    wv = nc.dram_tensor("wv", [D, HD], F32, kind="ExternalInput").ap()
    wo = nc.dram_tensor("wo", [DQ, D], F32, kind="ExternalInput").ap()
    cosT = nc.dram_tensor("cosT", [HD, T], F32, kind="ExternalInput").ap()
    sinT = nc.dram_tensor("sinT", [HD, T], F32, kind="ExternalInput").ap()
    maskb = nc.dram_tensor("maskb", [128, 896], F32, kind="ExternalInput").ap()
    ones = nc.dram_tensor("ones", [128, 2], F32, kind="ExternalInput").ap()
    outT = nc.dram_tensor("outT", [D, T], F32, kind="ExternalOutput").ap()

    with tile.TileContext(nc) as tc:
        _body(tc, outT, xT, wq, wk, wv, wo, cosT, sinT, maskb, ones)
    nc.compile()
    return nc


def _body(tc, outT, xT, wq, wk, wv, wo, cosT, sinT, maskb, ones):
    nc = tc.nc
    from contextlib import ExitStack

    with ExitStack() as ctx:
        const_pool = ctx.enter_context(tc.tile_pool(name="const", bufs=1))
        w_pool = ctx.enter_context(tc.tile_pool(name="wp", bufs=1))
        x_pool = ctx.enter_context(tc.tile_pool(name="xp", bufs=4))
        qt_pool = ctx.enter_context(tc.tile_pool(name="qtp", bufs=5))
        kt_pool = ctx.enter_context(tc.tile_pool(name="ktp", bufs=4))
        v_pool = ctx.enter_context(tc.tile_pool(name="vp", bufs=16))
        vt_pool = ctx.enter_context(tc.tile_pool(name="vtp", bufs=1))
        pt_pool = ctx.enter_context(tc.tile_pool(name="ptp", bufs=4))
        cx_pool = ctx.enter_context(tc.tile_pool(name="cxp", bufs=16))
        rope_pool = ctx.enter_context(tc.tile_pool(name="ropep", bufs=2))
        rb_pool = ctx.enter_context(tc.tile_pool(name="rbp", bufs=2))
        rc_pool = ctx.enter_context(tc.tile_pool(name="rcp", bufs=1))
        wo_pool = ctx.enter_context(tc.tile_pool(name="wop", bufs=2))
        o_pool = ctx.enter_context(tc.tile_pool(name="op", bufs=2))
        ps_pool = ctx.enter_context(tc.tile_pool(name="ps", bufs=8, space="PSUM"))

        # ---- constants (scalar-engine DGE so the sync queue starts on x) ----
        mask_sb = const_pool.tile([128, 896], F32, tag="mask")
        ones_sb = const_pool.tile([128, 2], F32R, tag="ones")
        ident_sb = const_pool.tile([128, 128], F32, tag="ident")
        nc.scalar.dma_start(mask_sb[:], maskb[:, :])
        nc.scalar.dma_start(ones_sb[:], ones[:, :].bitcast(F32R))
        make_identity(nc, ident_sb[:])
        cs_pool = ctx.enter_context(tc.tile_pool(name="csp", bufs=2))

        # ---- resident weights (f32r); loaded inside chunk-0 k-loop so the
        # sync queue serves the first matmuls' inputs immediately ----
        wq_sb = w_pool.tile([128, KT * DQ], F32R, tag="wq")
        wk_sb = w_pool.tile([128, KT * HD], F32R, tag="wk")
        wv_sb = w_pool.tile([128, KT * HD], F32R, tag="wv")

        kt_tiles = []      # kT chunk tiles [128, TC] (d x tokens), f32r
        v_tiles = []       # v j-tiles [128, 128] (tokens x d), f32r
        cx_tiles = {}      # (h, chunk) -> ctxT tile [128, TC], f32r

        for c in range(NCH):
            # ================= projections for token chunk c =================
            ps_q = [ps_pool.tile([128, TC], F32, tag="ps", name=f"psq{h}_{c}")
                    for h in range(NH)]
            ps_k = ps_pool.tile([128, TC], F32, tag="ps", name=f"psk_{c}")
            ps_v = ps_pool.tile([128, TC], F32, tag="ps", name=f"psv_{c}")
            for k in range(KT):
                if c == 0:
                    nc.sync.dma_start(
                        wq_sb[:, k * DQ:(k + 1) * DQ],
                        wq[k * 128:(k + 1) * 128, :].bitcast(F32R),
                    )
                    nc.sync.dma_start(
                        wk_sb[:, k * HD:(k + 1) * HD],
                        wk[k * 128:(k + 1) * 128, :].bitcast(F32R),
                    )
                    nc.sync.dma_start(
                        wv_sb[:, k * HD:(k + 1) * HD],
                        wv[k * 128:(k + 1) * 128, :].bitcast(F32R),
                    )
                xt = x_pool.tile([128, TC], F32R, tag="x", name=f"x_{c}_{k}")
                nc.sync.dma_start(
                    xt[:], xT[k * 128:(k + 1) * 128, c * TC:(c + 1) * TC].bitcast(F32R)
                )
                first, last = k == 0, k == KT - 1
                for h in range(NH):
                    nc.tensor.matmul(
                        ps_q[h][:],
                        wq_sb[:, k * DQ + h * HD:k * DQ + (h + 1) * HD],
                        xt[:],
                        start=first, stop=last,
                    )
                nc.tensor.matmul(
                    ps_k[:], wk_sb[:, k * HD:(k + 1) * HD], xt[:],
                    start=first, stop=last,
                )
                nc.tensor.matmul(
                    ps_v[:], wv_sb[:, k * HD:(k + 1) * HD], xt[:],
                    start=first, stop=last,
                )

            cs_t = cs_pool.tile([HD, TC], F32, tag="cos", name=f"cos_{c}")
            sn_t = cs_pool.tile([HD, TC], F32, tag="sin", name=f"sin_{c}")
            nc.scalar.dma_start(cs_t[:], cosT[:, c * TC:(c + 1) * TC])
            nc.scalar.dma_start(sn_t[:], sinT[:, c * TC:(c + 1) * TC])
            cs = cs_t[:, :]
            sn = sn_t[:, :]

            # rope on q heads -> qT chunk tiles (f32r)
            q_chunk = []
            for h in range(NH):
                t1 = rope_pool.tile([128, TC], F32, tag="t1", name=f"rq1_{c}_{h}")
                t2 = rope_pool.tile([128, TC], F32, tag="t2", name=f"rq2_{c}_{h}")
                nc.vector.tensor_mul(t1[:], ps_q[h][:], cs)
                nc.vector.tensor_mul(t2[0:64, :], ps_q[h][64:128, :], sn[0:64, :])
                nc.vector.tensor_mul(t2[64:128, :], ps_q[h][0:64, :], sn[64:128, :])
                qt = qt_pool.tile([128, TC], F32R, tag="qt", name=f"qt_{c}_{h}")
                nc.vector.tensor_add(qt[:], t1[:], t2[:])
                q_chunk.append(qt)

            # rope on k -> kT chunk tile (f32r)
            t1 = rope_pool.tile([128, TC], F32, tag="t1", name=f"rk1_{c}")
            t2 = rope_pool.tile([128, TC], F32, tag="t2", name=f"rk2_{c}")
            nc.vector.tensor_mul(t1[:], ps_k[:], cs)
            nc.vector.tensor_mul(t2[0:64, :], ps_k[64:128, :], sn[0:64, :])
            nc.vector.tensor_mul(t2[64:128, :], ps_k[0:64, :], sn[64:128, :])
            kt = kt_pool.tile([128, TC], F32R, tag="kt", name=f"kt_{c}")
            nc.vector.tensor_add(kt[:], t1[:], t2[:])
            kt_tiles.append(kt)

            # v: psum -> sbuf, then PE-transpose each [128,128] to tokens-major
            vt = vt_pool.tile([128, TC], F32, tag="vt", name=f"vt_{c}")
            nc.scalar.copy(vt[:], ps_v[:])
            for jj in range(TC // 128):
                ps_t = ps_pool.tile([128, 128], F32, tag="ps",
                                    name=f"pst_{c}_{jj}")
                nc.tensor.transpose(ps_t[:], vt[:, jj * 128:(jj + 1) * 128],
                                    ident_sb[:])
                vsb = v_pool.tile([128, 128], F32R, tag="v", name=f"v_{c}_{jj}")
                nc.vector.tensor_copy(vsb[:], ps_t[:])
                v_tiles.append(vsb)

            # ========== attention for i-chunk I = c, two heads at a time =====
            I = c
            nj = 4 * I + 4
            for hp in range(NH // 2):
                hs = [2 * hp, 2 * hp + 1]
                ps_ctx = {h: ps_pool.tile([128, TC], F32, tag="ps",
                                          name=f"psctx_{I}_{h}") for h in hs}
                ps_sum = {h: ps_pool.tile([2, TC], F32, tag="ps",
                                          name=f"pssum_{I}_{h}") for h in hs}
                for J in range(nj):
                    pts = {}
                    for h in hs:  # kT_J stationary shared across the pair
                        ps_s = ps_pool.tile([128, TC], F32, tag="ps",
                                            name=f"pss_{I}_{h}_{J}")
                        nc.tensor.matmul(
                            ps_s[:],
                            kt_tiles[J // 4][:, (J % 4) * 128:(J % 4 + 1) * 128],
                            q_chunk[h][:],
                            start=True, stop=True,
                        )
                        if J >= 4 * I:  # diagonal tile: additive causal mask
                            q = J - 4 * I
                            off = (3 - q) * 128
                            nc.vector.tensor_add(
                                ps_s[:], ps_s[:], mask_sb[:, off:off + TC]
                            )
                        pt = pt_pool.tile([128, TC], F32R, tag="pt",
                                          name=f"pt_{I}_{h}_{J}")
                        nc.scalar.activation(
                            pt[:], ps_s[:], mybir.ActivationFunctionType.Exp,
                            scale=SCALE,
                        )
                        pts[h] = pt
                    first, last = J == 0, J == nj - 1
                    for h in hs:  # v_J stationary shared across the pair
                        nc.tensor.matmul(ps_ctx[h][:], v_tiles[J][:], pts[h][:],
                                         start=first, stop=last)
                    for h in hs:  # ones stationary (trivial ldweights)
                        nc.tensor.matmul(ps_sum[h][:], ones_sb[:], pts[h][:],
                                         start=first, stop=last)

                for h in hs:
                    # free psum fast via ACT copies, then normalize out-of-band
                    sum_sb = rc_pool.tile([1, TC], F32, tag="recip",
                                          name=f"rc_{I}_{h}")
                    nc.scalar.copy(sum_sb[:], ps_sum[h][0:1, :])
                    cxt = cx_pool.tile([128, TC], F32R, tag="cx",
                                       name=f"cx_{I}_{h}")
                    nc.scalar.copy(cxt[:], ps_ctx[h][:])
                    rb = rb_pool.tile([128, TC], F32, tag="rb",
                                      name=f"rb_{I}_{h}")
                    nc.gpsimd.partition_broadcast(rb[:], sum_sb[:])
                    nc.vector.reciprocal(rb[:], rb[:])
                    nc.vector.tensor_mul(cxt[:], cxt[:], rb[:])
                    cx_tiles[(h, I)] = cxt

        # ======= output stage: outT[e,t], wo-tile stationary reused 4x =======
        for Et in range(NET):
            woe = wo_pool.tile([128, NH * 128], F32R, tag="wo", name=f"wo_{Et}")
            for h in range(NH):
                nc.sync.dma_start(
                    woe[:, h * 128:(h + 1) * 128],
                    wo[h * HD:(h + 1) * HD, Et * 128:(Et + 1) * 128].bitcast(F32R),
                )
            ps_o = [ps_pool.tile([128, TC], F32, tag="ps", name=f"pso_{Et}_{tc_}")
                    for tc_ in range(NCH)]
            for h in range(NH):
                for tc_ in range(NCH):
                    nc.tensor.matmul(
                        ps_o[tc_][:],
                        woe[:, h * 128:(h + 1) * 128],
                        cx_tiles[(h, tc_)][:],
                        start=h == 0, stop=h == NH - 1,
                    )
            for tc_ in range(NCH):
                ot = o_pool.tile([128, TC], F32, tag="o", name=f"o_{Et}_{tc_}")
                if tc_ % 2 == 0:
                    nc.vector.tensor_copy(ot[:], ps_o[tc_][:])
                else:
                    nc.scalar.copy(ot[:], ps_o[tc_][:])
                nc.sync.dma_start(
                    outT[Et * 128:(Et + 1) * 128, tc_ * TC:(tc_ + 1) * TC], ot[:]
                )


# ---------------------------------------------------------------------------
# host side
# ---------------------------------------------------------------------------
_NC_CACHE = None


def _get_nc():
    global _NC_CACHE
    if _NC_CACHE is None:
        _NC_CACHE = build_nc()
    return _NC_CACHE


def make_in_maps(x, Wq, Wk, Wv, Wo, cos, sin):
    x = np.asarray(x, dtype=np.float32)
    xT = np.ascontiguousarray(x.reshape(T, D).T)
    cosT = np.ascontiguousarray(np.asarray(cos, np.float32)[:T].T)
    sin_t = np.asarray(sin, np.float32)[:T]          # [T, 128]
    sinT = sin_t.T.copy()                            # [128, T]
    sinT[:64] *= -1.0                                # fold rotate-half sign
    sinT = np.ascontiguousarray(sinT)

    # sliding additive causal mask: tile q reads cols (3-q)*128 : (3-q)*128+512
    # of big[r, cc] = 0 if cc >= 384 + r else -1e30
    r = np.arange(128)[:, None]
    cc = np.arange(896)[None, :]
    m = np.where(cc >= 384 + r, 0.0, -1.0e30).astype(np.float32)
    ones = np.ones((128, 2), np.float32)

    Wq = np.asarray(Wq, np.float32)
    Wk = np.asarray(Wk, np.float32)
    Wv = np.asarray(Wv, np.float32)
    Wo = np.asarray(Wo, np.float32)
    in_maps = []
    for g in range(NCORES):
        in_maps.append({
            "xT": xT,
            "wq": np.ascontiguousarray(Wq[:, g * DQ:(g + 1) * DQ]),
            "wk": np.ascontiguousarray(Wk[:, g * HD:(g + 1) * HD]),
            "wv": np.ascontiguousarray(Wv[:, g * HD:(g + 1) * HD]),
            "wo": np.ascontiguousarray(Wo[g * DQ:(g + 1) * DQ, :]),
            "cosT": cosT,
            "sinT": sinT,
            "maskb": m,
            "ones": ones,
        })
    return in_maps


def kernel(x, Wq, Wk, Wv, Wo, cos, sin):
    nc = _get_nc()
    in_maps = make_in_maps(x, Wq, Wk, Wv, Wo, cos, sin)
    res = run_bass_kernel_spmd(nc, in_maps, core_ids=list(range(NCORES)))
    acc = np.zeros((D, T), np.float32)
    for c in range(NCORES):
        acc += res.results[c]["outT"]
    return np.ascontiguousarray(acc.T).reshape(1, T, D)
